# revision 31
# baseline (speedup 1.0000x reference)
"""Trainium2 Bass kernel for nn_CapsuleModel2 (capsule routing head).

Strategy (data-parallel, one image per NeuronCore, 8 cores):

Host-side algebraic folding:
  The whole per-pixel chain  1x1conv(poses) -> per-capsule vote conv ->
  positional-encoding linear  collapses into a single effective matmul:
     tokens_grid[(n,v), s] = Weff @ feat + (r(s)*w_d + b_eff)
  where Weff = W16 @ w_vote[n] @ w_poses[n]  (host-precomputed, 128x1280)
  and the positional encoding is rank-1 in the *grid position only*:
  pe = [(y-x)/128, (x-y)/128] so pe @ w_pos[:,16:18].T = r(s) * (wy-wx).
  That grid-constant [128,4096] table ships from the host.

Device pipeline per core (one image), per repetition:
  G. tokens_grid = WeffT.T @ feat (bf16 matmul, fp32 psum) + PEGRID  [128,4096]
     z_grid = w_acts @ feat + b_acts (activation logits)             [8,4096]
  H. ap_gather (GPSIMD ucode) pulls the I*P=4096 point columns:
     tok_all[(n,v), (i,p)] (fp32, converted to bf16 per quarter),
     zg[(n), (i,p)] -> sigmoid+1e-6 once on [128,1024] (bf16)
  R1. per 128-point chunk, all-bf16 matmuls:
     LT[pt,(n,o)] = tok_chunk.T @ blockdiag(Q1/4)        E = exp(LT) bf16
     vals[pt,(n,j)] = asig * (tok_chunk.T @ blockdiag(Wv1) | 1)      bf16
     numer accumulates TRANSPOSED per instance pair:
        psum[(ipar*64+o), j(17)] += E_n.T @ vals_n      (2 inst / psum tile)
  R2. per instance pair (o on partitions -> normalization is per-partition):
     p1 = numer[:, :16] / numer[:, 16];  a1 = sigmoid(p1 . wact1 + b)
     pv2 = (a1+1e-6) * (p1 | 1);  p1T via PE transpose;  E2 = exp(p1T.T @ Q2s)
     pnd_all[17, 19*inst] += pv2_h.T @ E2_h   (one shared psum tile)
  Tail (once): class outputs for all 16 instances in one [1,304] chain:
     out = sigmoid((wact2ext.T @ pnd) / denom + b)
"""

import sys

for _p in ("/opt/trn_rl_repo",):
    if _p not in sys.path:
        sys.path.insert(0, _p)

import numpy as np
import ml_dtypes

import concourse.bacc as bacc
import concourse.tile as tile
from concourse import mybir
from concourse import bass_utils

AF = mybir.ActivationFunctionType
PM_DR = mybir.MatmulPerfMode.DoubleRow
ALU = mybir.AluOpType
F32 = mybir.dt.float32
FP8 = mybir.dt.float8e4
BF16 = mybir.dt.bfloat16
I16 = mybir.dt.int16
BF16_NP = ml_dtypes.bfloat16
FP8_NP = ml_dtypes.float8_e4m3
WSCALE = 256.0

B, I, P = 8, 16, 256
CIN = 1280
NCAPS, DCAP, DV = 8, 32, 16
HF = WF = 64
S = HF * WF              # 4096 grid positions
NPTS = I * P             # 4096 gathered points
NOUT1, NCLS = 64, 19
KT = CIN // 128          # 10 contraction tiles
HALF = S // 2
ZW = S + 8               # z grid padded with a -inf slot for masked points
NCH = 32                 # routing-1 chunks of 128 points
QIDX = NPTS // 4         # points per gather quarter
NCLS2 = 20               # class dim padded to 8-byte psum alignment

_CACHE = {}


def _build_nc(phases=4, repeat=1, r2stop=7):
    # phases: 1 = grid matmuls only, 2 = +gathers, 3 = +routing1, 4 = +routing2
    nc = bacc.Bacc("TRN2", target_bir_lowering=False, debug=False, num_devices=8)

    din = {}

    def dram_in(name, shape, dt):
        din[name] = nc.dram_tensor(name, list(shape), dt, kind="ExternalInput").ap()
        return din[name]

    feat = dram_in("feat", (CIN, S), FP8)
    pegrid = dram_in("pegrid", (128, S), F32)
    weffT = dram_in("weffT", (CIN, 128), FP8)
    waT = dram_in("waT", (CIN, 8), FP8)
    bacts = dram_in("bacts", (8, 1), F32)
    bq1 = dram_in("bq1", (128, 512), BF16)
    bwv1 = dram_in("bwv1", (128, 136), BF16)
    exp8rep = dram_in("exp8rep", (128, 136), BF16)
    q2sT = dram_in("q2sT", (16, NCLS2), BF16)
    wact1rep = dram_in("wact1rep", (128, 16), BF16)
    nbact1rep = dram_in("nbact1rep", (128, 1), F32)
    w2ext = dram_in("w2ext", (17, 2), F32)
    nbact2s = dram_in("nbact2s", (1, 1), F32)
    identb = dram_in("identb", (128, 128), BF16)
    gidx = dram_in("gidx", (128, NPTS // 16), I16)
    aidx = dram_in("aidx", (128, NPTS // 64), I16)

    out_cls = nc.dram_tensor("out_cls", [I, NCLS], F32, kind="ExternalOutput").ap()

    with tile.TileContext(nc) as tc:
        with (
            tc.tile_pool(name="cons", bufs=1) as cons,
            tc.tile_pool(name="grid", bufs=2) as grid,
            tc.tile_pool(name="grid1", bufs=1) as grid1,
            tc.tile_pool(name="feats", bufs=4) as feats,
            tc.tile_pool(name="idxp", bufs=2) as idxp,
            tc.tile_pool(name="rsb", bufs=3) as rsb,
            tc.tile_pool(name="small", bufs=2) as small,
            # Static PSUM partition: grid gets 2 banks, routing gets 6, so
            # rep r+1's grid matmuls never wait on rep r's routing reads.
            tc.tile_pool(name="pgp", bufs=3, space="PSUM") as pgp,
            tc.tile_pool(name="plp", bufs=2, space="PSUM") as plp,
            tc.tile_pool(name="pvap", bufs=1, space="PSUM") as pvap,
            tc.tile_pool(name="pnp", bufs=1, space="PSUM") as pnp,
            tc.tile_pool(name="pr2p", bufs=1, space="PSUM") as pr2p,
        ):
            # ---- constants to SBUF (once) ----
            pegrid_sb = cons.tile([128, S], F32)
            for jq in range(4):
                nc.sync.dma_start(
                    out=pegrid_sb[:, jq * 1024 : (jq + 1) * 1024],
                    in_=pegrid[:, jq * 1024 : (jq + 1) * 1024],
                )
            weffT_sb = cons.tile([128, KT // 2, 2, 128], FP8)
            nc.sync.dma_start(
                out=weffT_sb[:],
                in_=weffT.rearrange("(kk two p) m -> p kk two m", p=128, two=2),
            )
            waT_sb = cons.tile([128, KT, 8], FP8)
            nc.sync.dma_start(
                out=waT_sb[:], in_=waT.rearrange("(k p) m -> p k m", p=128)
            )
            bacts_sb = cons.tile([8, 1], F32)
            nc.sync.dma_start(out=bacts_sb[:], in_=bacts)
            bq1_sb = cons.tile([128, 512], BF16)
            nc.sync.dma_start(out=bq1_sb[:], in_=bq1)
            bwv1_sb = cons.tile([128, 136], BF16)
            nc.sync.dma_start(out=bwv1_sb[:], in_=bwv1)
            exp8rep_sb = cons.tile([128, 136], BF16)
            nc.sync.dma_start(out=exp8rep_sb[:], in_=exp8rep)
            q2sT_sb = cons.tile([16, NCLS2], BF16)
            nc.sync.dma_start(out=q2sT_sb[:], in_=q2sT)
            wact1rep_sb = cons.tile([128, 16], BF16)
            nc.sync.dma_start(out=wact1rep_sb[:], in_=wact1rep)
            nbact1rep_sb = cons.tile([128, 1], F32)
            nc.sync.dma_start(out=nbact1rep_sb[:], in_=nbact1rep)
            w2ext_sb = cons.tile([17, 2], F32)
            nc.sync.dma_start(out=w2ext_sb[:], in_=w2ext)
            nbact2s_sb = cons.tile([1, 1], F32)
            nc.sync.dma_start(out=nbact2s_sb[:], in_=nbact2s)
            identb_sb = cons.tile([128, 128], BF16)
            nc.sync.dma_start(out=identb_sb[:], in_=identb)

            # z grid replicated at 32-partition strides so one ap_gather can
            # split the 4096 points across 4 GPSIMD core pairs.
            z_rep = cons.tile([128, ZW], F32)
            nc.vector.memset(z_rep[:, :], 0.0)
            nc.vector.memset(z_rep[0:8, S:ZW], -10000.0)

            for rep in range(repeat):
                # ---- per-rep input loads (points / indices) ----
                gidx_sb = idxp.tile([128, NPTS // 16], I16, tag="gidx", name=f"gi{rep}")
                nc.scalar.dma_start(out=gidx_sb[:], in_=gidx)
                aidx_sb = idxp.tile([128, NPTS // 64], I16, tag="aidx", name=f"ai{rep}")
                nc.scalar.dma_start(out=aidx_sb[:], in_=aidx)

                feat_sb = grid1.tile([128, KT * S], FP8, tag="f8", name=f"f8{rep}")
                nc.sync.dma_start(
                    out=feat_sb[:].rearrange("p (k s) -> p k s", k=KT),
                    in_=feat.rearrange("(k p) s -> p k s", p=128),
                )
                featv = feat_sb[:].rearrange("p (k s) -> p k s", k=KT)
                tokens_sb = grid.tile([128, S], F32, tag="tok", name=f"tok{rep}")
                tok_all = grid1.tile([128, NPTS], F32, tag="ta", name=f"ta{rep}")
                tok_all_b = grid.tile([128, NPTS], BF16, tag="tab", name=f"tab{rep}")
                zg2 = grid1.tile([128, NPTS // 4], F32, tag="zg", name=f"zg{rep}")
                zg2s = grid.tile([128, NPTS // 4], BF16, tag="zgs", name=f"zgs{rep}")

                # ---- phase G: grid matmuls (fp8, SBUF-resident feat, 4 banks) ----
                for pss in range(8):
                    off = pss * 512
                    pm = pgp.tile([128, 512], F32, tag="gm", name=f"pm{pss}_{rep}")
                    for kk in range(KT // 2):
                        nc.tensor.matmul(
                            pm[:],
                            lhsT=weffT_sb[:, kk, :, :],
                            rhs=featv[:, 2 * kk : 2 * kk + 2, off : off + 512],
                            start=(kk == 0),
                            stop=(kk == KT // 2 - 1),
                            perf_mode=PM_DR,
                        )
                    nc.vector.scalar_tensor_tensor(
                        out=tokens_sb[:, off : off + 512],
                        in0=pm[:],
                        scalar=1.0 / WSCALE,
                        in1=pegrid_sb[:, off : off + 512],
                        op0=ALU.mult,
                        op1=ALU.add,
                    )
                    pa = pgp.tile([128, 512], F32, tag="gm", name=f"pa{pss}_{rep}")
                    for k in range(KT):
                        nc.tensor.matmul(
                            pa[0:8, :],
                            lhsT=waT_sb[:, k, :],
                            rhs=featv[:, k, off : off + 512],
                            start=(k == 0),
                            stop=(k == KT - 1),
                        )
                    nc.scalar.activation(
                        out=z_rep[0:8, off : off + 512],
                        in_=pa[0:8, :],
                        func=AF.Identity,
                        bias=bacts_sb[:],
                        scale=1.0 / WSCALE,
                    )

                if phases < 4 or r2stop < 7:
                    orow0 = small.tile([1, I * NCLS2], F32, tag="orow", name=f"or{rep}")
                    nc.vector.memset(orow0[:, :], 0.0)
                    nc.scalar.dma_start(
                        out=out_cls.rearrange("(a i) c -> a i c", a=1),
                        in_=orow0[:].rearrange("p (i c) -> p i c", c=NCLS2)[:, :, 0:NCLS],
                    )

                if phases < 2:
                    continue

                # replicate z rows 0:8 to partition bases 32/64/96
                for mq in range(1, 4):
                    nc.scalar.dma_start(
                        out=z_rep[32 * mq : 32 * mq + 8, :], in_=z_rep[0:8, :]
                    )

                # ---- phase H: gathers (GPSIMD ucode) ----
                nc.gpsimd.ap_gather(
                    zg2[:],
                    z_rep[:],
                    aidx_sb[:],
                    channels=128,
                    num_elems=ZW,
                    d=1,
                    num_idxs=QIDX,
                )
                # asig = sigmoid(z) + 1e-6, once, in bf16 (in place on zg2)
                nc.scalar.activation(out=zg2[:], in_=zg2[:], func=AF.Exp, scale=-1.0)
                nc.vector.tensor_scalar_add(out=zg2[:], in0=zg2[:], scalar1=1.0)
                nc.vector.reciprocal(out=zg2[:], in_=zg2[:])
                nc.vector.tensor_scalar_add(out=zg2s[:], in0=zg2[:], scalar1=1e-6)

                for q in range(4):
                    nc.gpsimd.ap_gather(
                        tok_all[:, q * QIDX : (q + 1) * QIDX],
                        tokens_sb[:],
                        gidx_sb[:, q * (QIDX // 16) : (q + 1) * (QIDX // 16)],
                        channels=128,
                        num_elems=S,
                        d=1,
                        num_idxs=QIDX,
                    )
                    nc.vector.tensor_copy(
                        out=tok_all_b[:, q * QIDX : (q + 1) * QIDX],
                        in_=tok_all[:, q * QIDX : (q + 1) * QIDX],
                    )

                if phases < 3:
                    continue

                # ---- phase R1 + R2 ----
                # Software-pipelined: chunk c's numer matmuls are emitted after
                # chunk c+1's front so the PE never stalls on post-processing.
                if True:
                    pndsb = small.tile(
                        [17, I * NCLS2], F32, tag="pndsb", name=f"ps{rep}"
                    )
                    state = {"pn": None}

                    def emit_front(c):
                        tokc = tok_all_b[:, c * 128 : (c + 1) * 128]
                        pl = plp.tile([128, 512], F32, tag="pl", name=f"pl{c}_{rep}")
                        nc.tensor.matmul(
                            pl[:], lhsT=tokc, rhs=bq1_sb[:], start=True, stop=True
                        )
                        E = rsb.tile([128, 512], BF16, tag="E", name=f"E{c}_{rep}")
                        nc.scalar.activation(out=E[:], in_=pl[:], func=AF.Exp)

                        # pv (cols 0:136) and replicated asig (cols 136:272)
                        # share one psum bank
                        pva = pvap.tile([128, 272], F32, tag="pva", name=f"pva{c}_{rep}")
                        nc.tensor.matmul(
                            pva[:, 0:136], lhsT=tokc, rhs=bwv1_sb[:], start=True,
                            stop=True, skip_group_check=True,
                        )
                        m4 = c // 8
                        nc.tensor.matmul(
                            pva[:, 136:272],
                            lhsT=zg2s[
                                32 * m4 : 32 * m4 + 8,
                                (c % 8) * 128 : (c % 8 + 1) * 128,
                            ],
                            rhs=exp8rep_sb[32 * m4 : 32 * m4 + 8, :],
                            start=True,
                            stop=True,
                            tile_position=(32 * m4, 0),
                            skip_group_check=True,
                        )
                        asigs = rsb.tile([128, 136], BF16, tag="asigs", name=f"ag{c}_{rep}")
                        nc.vector.tensor_copy(out=asigs[:], in_=pva[:, 136:272])
                        vals = rsb.tile([128, 136], BF16, tag="vals", name=f"va{c}_{rep}")
                        nc.vector.tensor_mul(
                            out=vals[:], in0=asigs[:], in1=pva[:, 0:136]
                        )
                        # the 17th column of each block is asig itself (denom)
                        vr = vals[:].rearrange("p (n j) -> p n j", j=17)
                        ar = asigs[:].rearrange("p (n j) -> p n j", j=17)
                        nc.vector.tensor_copy(
                            out=vr[:, :, 16:17], in_=ar[:, :, 16:17]
                        )
                        return E, vals

                    def emit_numer(c, E, vals):
                        # instance i = c//2; pair = i//2; half = i%2
                        i = c // 2
                        half = i % 2
                        if c % 4 == 0:
                            state["pn"] = pnp.tile(
                                [128, 17], F32, tag="pn", name=f"pn{c // 4}_{rep}"
                            )
                        pn = state["pn"]
                        first = c % 2 == 0
                        for n in range(8):
                            nc.tensor.matmul(
                                pn[64 * half : 64 * half + 64, :],
                                lhsT=E[:, n * 64 : (n + 1) * 64],
                                rhs=vals[:, n * 17 : (n + 1) * 17],
                                start=(first and n == 0),
                                stop=((not first) and n == 7),
                                skip_group_check=True,
                            )
                        return pn

                    def emit_r2(pair, pn):
                        # pn [128, 17]: two instances' numer, o on partitions
                        recd = small.tile([128, 1], F32, tag="recd", name=f"rc{pair}_{rep}")
                        nc.vector.reciprocal(out=recd[:], in_=pn[:, 16:17])
                        p1i = small.tile([128, 16], BF16, tag="p1i", name=f"p1i{pair}_{rep}")
                        nc.vector.tensor_scalar_mul(
                            out=p1i[:], in0=pn[:, 0:16], scalar1=recd[:]
                        )
                        if r2stop < 2:
                            return
                        # a1 = sigmoid(p1 . wact1 + b) + 1e-6
                        z1t = small.tile([128, 16], BF16, tag="z1t", name=f"z1t{pair}_{rep}")
                        nc.vector.tensor_mul(out=z1t[:], in0=p1i[:], in1=wact1rep_sb[:])
                        a1 = small.tile([128, 1], F32, tag="a1", name=f"a1_{pair}_{rep}")
                        nc.vector.reduce_sum(out=a1[:], in_=z1t[:], axis=mybir.AxisListType.X)
                        nc.scalar.activation(
                            out=a1[:], in_=a1[:], func=AF.Exp, scale=-1.0,
                            bias=nbact1rep_sb[:],
                        )
                        nc.vector.tensor_scalar_add(out=a1[:], in0=a1[:], scalar1=1.0)
                        nc.vector.reciprocal(out=a1[:], in_=a1[:])
                        nc.vector.tensor_scalar_add(out=a1[:], in0=a1[:], scalar1=1e-6)
                        if r2stop < 3:
                            return
                        pv2 = small.tile([128, 17], BF16, tag="pv2", name=f"pv2_{pair}_{rep}")
                        nc.vector.tensor_scalar_mul(
                            out=pv2[:, 0:16], in0=p1i[:], scalar1=a1[:]
                        )
                        nc.vector.tensor_copy(out=pv2[:, 16:17], in_=a1[:])
                        if r2stop < 4:
                            return
                        # p1T: [16, 128] via PE transpose
                        pT = pr2p.tile([16, 128], BF16, tag="r2x", name=f"pT{pair}_{rep}")
                        nc.tensor.transpose(
                            out=pT[:], in_=p1i[:], identity=identb_sb[:]
                        )
                        p1T = small.tile([16, 128], BF16, tag="p1T", name=f"p1T{pair}_{rep}")
                        nc.vector.tensor_copy(out=p1T[:], in_=pT[:])
                        if r2stop < 5:
                            return
                        pL2 = pr2p.tile([128, NCLS2], F32, tag="r2x", name=f"pL2{pair}_{rep}")
                        nc.tensor.matmul(
                            pL2[:], lhsT=p1T[:], rhs=q2sT_sb[:], start=True, stop=True
                        )
                        E2 = small.tile([128, NCLS2], BF16, tag="E2", name=f"E2_{pair}_{rep}")
                        nc.scalar.activation(out=E2[:], in_=pL2[:], func=AF.Exp)
                        if r2stop < 6:
                            return
                        for h2 in range(2):
                            inst = 2 * pair + h2
                            pnd_i = pr2p.tile(
                                [17, NCLS2], F32, tag="r2x", name=f"pi{inst}_{rep}"
                            )
                            nc.tensor.matmul(
                                pnd_i[:],
                                lhsT=pv2[64 * h2 : 64 * h2 + 64, :],
                                rhs=E2[64 * h2 : 64 * h2 + 64, :],
                                start=True,
                                stop=True,
                            )
                            nc.vector.tensor_copy(
                                out=pndsb[:, NCLS2 * inst : NCLS2 * (inst + 1)],
                                in_=pnd_i[:],
                            )

                    pending = None
                    for c in range(NCH):
                        front = emit_front(c)
                        if pending is not None:
                            pc, pE, pvals = pending
                            pn = emit_numer(pc, pE, pvals)
                            if pc % 4 == 3 and phases >= 4:
                                emit_r2(pc // 4, pn)
                        pending = (c,) + front
                    pc, pE, pvals = pending
                    pn = emit_numer(pc, pE, pvals)
                    if phases >= 4:
                        emit_r2(pc // 4, pn)

                    if phases >= 4 and r2stop >= 7:
                        # ---- tail: all 16 instances at once ----
                        wdp = pr2p.tile([1, I * NCLS2], F32, tag="r2x", name=f"wd{rep}")
                        nc.tensor.matmul(
                            wdp[:], lhsT=w2ext_sb[:, 0:1], rhs=pndsb[:], start=True,
                            stop=True,
                        )
                        # denominator row lifted to partition 0 via one-hot matmul
                        den0 = pnp.tile([1, I * NCLS2], F32, tag="pn", name=f"dn{rep}")
                        nc.tensor.matmul(
                            den0[:], lhsT=w2ext_sb[:, 1:2], rhs=pndsb[:], start=True,
                            stop=True,
                        )
                        rec2 = small.tile([1, I * NCLS2], F32, tag="rec2", name=f"r2_{rep}")
                        nc.vector.reciprocal(out=rec2[:], in_=den0[:])
                        z2m = small.tile([1, I * NCLS2], F32, tag="z2m", name=f"z2{rep}")
                        nc.vector.tensor_mul(out=z2m[:], in0=wdp[:], in1=rec2[:])
                        nc.scalar.activation(
                            out=z2m[:], in_=z2m[:], func=AF.Exp, scale=-1.0,
                            bias=nbact2s_sb[:],
                        )
                        nc.vector.tensor_scalar_add(out=z2m[:], in0=z2m[:], scalar1=1.0)
                        orow = small.tile([1, I * NCLS2], F32, tag="orow", name=f"or{rep}")
                        nc.vector.reciprocal(out=orow[:], in_=z2m[:])
                        nc.scalar.dma_start(
                            out=out_cls.rearrange("(a i) c -> a i c", a=1),
                            in_=orow[:].rearrange("p (i c) -> p i c", c=NCLS2)[:, :, 0:NCLS],
                        )

    nc.compile()
    return nc


def _get_nc():
    if "nc" not in _CACHE:
        _CACHE["nc"] = _build_nc()
    return _CACHE["nc"]


def _wrap_idx(sidx):
    # ap_gather index layout: index j lives at partition j%16, column j//16.
    return np.ascontiguousarray(sidx.reshape(-1, 16).T.astype(np.int16))


def host_prep(inputs):
    """Build the per-core input maps (all numpy, host-side weight folding)."""
    f8 = np.float64
    w_pos = np.asarray(inputs["w_pos"], f8)          # (16, 18)
    W16 = w_pos[:, :16]
    w_d = w_pos[:, 16] - w_pos[:, 17]                # (16,)
    b_pos = np.asarray(inputs["b_pos"], f8)
    w_vote = np.asarray(inputs["w_vote"], f8)        # (8, 16, 32)
    b_vote = np.asarray(inputs["b_vote"], f8)        # (8, 16)
    Wp = np.asarray(inputs["w_poses"], f8).reshape(NCAPS, DCAP, CIN)
    b_poses = np.asarray(inputs["b_poses"], f8).reshape(NCAPS, DCAP)

    Weff = np.stack([W16 @ w_vote[n] @ Wp[n] for n in range(NCAPS)])  # (8,16,1280)
    beff = np.stack(
        [W16 @ (w_vote[n] @ b_poses[n] + b_vote[n]) + b_pos for n in range(NCAPS)]
    )                                                                  # (8,16)
    Weff = Weff.reshape(128, CIN)
    beff = beff.reshape(128)

    ss = np.arange(S)
    r = ((ss // WF) - (ss % WF)) / 128.0
    pegrid = (np.tile(w_d, NCAPS)[:, None] * r[None, :] + beff[:, None]).astype(
        np.float32
    )

    Q1s = np.asarray(inputs["Q1"], f8) / 4.0         # (64, 16)
    BQ1 = np.zeros((128, 512), np.float32)
    for n in range(NCAPS):
        BQ1[n * 16 : (n + 1) * 16, n * 64 : (n + 1) * 64] = Q1s.T
    Wv1 = np.asarray(inputs["Wv1"], f8)
    BWV1 = np.zeros((128, 136), np.float32)
    for n in range(NCAPS):
        BWV1[n * 16 : (n + 1) * 16, n * 17 : n * 17 + 16] = Wv1
    EXP8REP = np.zeros((128, 136), np.float32)
    for m in range(4):
        for n in range(NCAPS):
            EXP8REP[32 * m + n, n * 17 : (n + 1) * 17] = 1.0

    w2e = np.zeros((17, 2), np.float32)
    w2e[0:16, 0] = np.asarray(inputs["wact2"], np.float32)
    w2e[16, 1] = 1.0

    consts = dict(
        pegrid=pegrid,
        weffT=np.ascontiguousarray(Weff.T * WSCALE).astype(FP8_NP),
        waT=np.ascontiguousarray(
            np.asarray(inputs["w_acts"], f8).T * WSCALE
        ).astype(FP8_NP),
        bacts=np.asarray(inputs["b_acts"], np.float32).reshape(8, 1),
        bq1=BQ1.astype(BF16_NP),
        bwv1=BWV1.astype(BF16_NP),
        exp8rep=EXP8REP.astype(BF16_NP),
        q2sT=np.ascontiguousarray(
            np.concatenate(
                [(np.asarray(inputs["Q2"], f8) / 4.0).T, np.zeros((16, 1), f8)], axis=1
            )
        ).astype(BF16_NP),
        wact1rep=np.tile(
            np.asarray(inputs["wact1"], np.float32).reshape(1, 16), (128, 1)
        ).astype(BF16_NP),
        nbact1rep=np.full(
            (128, 1), -float(np.asarray(inputs["bact1"])), np.float32
        ),
        w2ext=w2e,
        nbact2s=np.full((1, 1), -float(np.asarray(inputs["bact2"])), np.float32),
        identb=np.eye(128, dtype=np.float32).astype(BF16_NP),
    )

    feats = np.asarray(inputs["feature_output"])     # (8, 1280, 64, 64) f32
    coords = np.asarray(inputs["point_coords"])      # (8, 16, 2, 256) int32
    mask = np.asarray(inputs["point_mask"])          # (8, 16, 256) bool

    in_maps = []
    for b in range(B):
        y = np.clip(coords[b, :, 0, :], 0, HF - 1).astype(np.int64)
        x = np.clip(coords[b, :, 1, :], 0, WF - 1).astype(np.int64)
        sidx = (y * WF + x).reshape(NPTS)
        zidx = sidx.copy()
        mb = mask[b].reshape(NPTS)
        zidx[~mb] = S  # masked points read the -1e4 z slot -> ~zero weight
        m = dict(consts)
        m["feat"] = np.ascontiguousarray(
            feats[b].reshape(CIN, S).astype(FP8_NP)
        )
        m["gidx"] = np.tile(_wrap_idx(sidx), (8, 1))
        # z-gather: GPSIMD core pair m handles point quarter m
        aidx = np.zeros((128, NPTS // 64), np.int16)
        for mq in range(4):
            aidx[32 * mq : 32 * mq + 16, :] = _wrap_idx(
                zidx[mq * (NPTS // 4) : (mq + 1) * (NPTS // 4)]
            )
        m["aidx"] = aidx
        in_maps.append(m)
    return in_maps


def kernel(**inputs):
    nc = _get_nc()
    in_maps = host_prep(inputs)
    res = bass_utils.run_bass_kernel_spmd(nc, in_maps, core_ids=list(range(B)))
    out = np.stack([np.asarray(res.results[b]["out_cls"]) for b in range(B)])
    return out.astype(np.float32)
